# revision 26
# baseline (speedup 1.0000x reference)
"""Trainium2 Bass kernel for nn_BrainRegion (liquid-gated recurrent cell).

Computes, for full inputs (B=8192, IN=H=2048):
    xin  = concat([x_t, state], -1)
    cand = tanh(xin @ Wc + state @ Uc + bc)
    gate = sigmoid(xin @ Wg + state @ Ug + bg)
    alpha = exp(-1/exp(log_step))
    h    = alpha * state + (1 - alpha) * gate * cand
    out  = layernorm(h) * gamma + beta

Strategy: data-parallel over batch across 8 NeuronCores (1024 rows/core),
weights replicated.  Algebraic fold: xin@Wc + state@Uc == x_t@Wc[:IN] +
state@(Wc[IN:] + Uc), which removes one third of the FLOPs.

Precision: mixed fp8/bf16 matmuls.  The x-path (both branches) and the
state->gate path run as fp8e4 DoubleRow matmuls (2 fp8 weights per PE
cell -> 2x tensor-engine throughput); the state->cand path stays bf16
because tanh' is ~2x steeper than sigmoid' so that path dominates the
error budget.  Operands are pre-scaled on host (x*16, W*512) so fp8e4
values sit well inside the normal range; the common 1/8192 descale is
folded into the epilogue activation scale.  fp32 PSUM accumulation;
elementwise epilogue + layernorm in fp32.  All matmuls use a 512-wide
moving operand (one full PSUM bank) to amortize per-instruction
overhead.
"""

import sys

if "/opt/trn_rl_repo" not in sys.path:
    sys.path.insert(0, "/opt/trn_rl_repo")

import numpy as np
import ml_dtypes

B, IN, H = 8192, 2048, 2048
NCORES = 8
BC = B // NCORES      # rows per core (1024)
P = 128               # partitions
G = BC // P           # batch groups per core (8)
NJ = 4                # H slices
NSL = H // NJ         # slice width (512)
KT = H // P           # k-tiles per matrix (16)
KP = KT // 2          # k-tile pairs for DoubleRow (8)
EPS = 1e-5

SX = 16.0             # activation quant scale (fp8 and bf16 copies)
SW = 512.0            # weight quant scale
INV = 1.0 / (SX * SW)

DVE_RSQRT = True      # rstd via DVE quake-rsqrt (else ScalarE Sqrt)

bf16 = ml_dtypes.bfloat16
f8e4 = ml_dtypes.float8_e4m3   # TRN FP8_EXP4-compatible (max 240)

# Set by test.py to collect a hardware profile.
TRACE = False
LAST_RESULTS = None

_compiled = {}


def _build(flags):
    """Trace + compile the SPMD device program. flags = (has_bc, has_bg,
    has_gamma, has_beta) selects optional elementwise passes."""
    from contextlib import ExitStack

    import concourse.bass as bass
    import concourse.tile as tile
    from concourse import bacc, mybir

    has_bc, has_bg, has_gamma, has_beta = flags
    f32 = mybir.dt.float32
    u32 = mybir.dt.uint32
    bft = mybir.dt.bfloat16
    f8t = mybir.dt.float8e4
    AF = mybir.ActivationFunctionType
    OP = mybir.AluOpType
    DR = mybir.MatmulPerfMode.DoubleRow

    nc = bacc.Bacc("TRN2", target_bir_lowering=False, debug=False,
                   num_devices=NCORES)

    # DRAM I/O. Activation tensors are pre-arranged on host so every DMA
    # below is contiguous:
    #   x8/s8/sb: [G, P, KT, P]      [g,p,k,m] = act[g*128+m, k*128+p]
    #   w*:       [NJ, P, KT, NSL]   [j,p,k,n] = W[k*128+p, j*NSL+n]
    x8 = nc.dram_tensor("x8", [G, P, KT, P], f8t, kind="ExternalInput").ap()
    s8 = nc.dram_tensor("s8", [G, P, KT, P], f8t, kind="ExternalInput").ap()
    sb = nc.dram_tensor("sb", [G, P, KT, P], bft, kind="ExternalInput").ap()
    st = nc.dram_tensor("st", [BC, H], bft, kind="ExternalInput").ap()
    wcx8 = nc.dram_tensor("wcx8", [NJ, P, KT, NSL], f8t,
                          kind="ExternalInput").ap()
    wgx8 = nc.dram_tensor("wgx8", [NJ, P, KT, NSL], f8t,
                          kind="ExternalInput").ap()
    wgs8 = nc.dram_tensor("wgs8", [NJ, P, KT, NSL], f8t,
                          kind="ExternalInput").ap()
    wcs = nc.dram_tensor("wcs", [NJ, P, KT, NSL], bft,
                         kind="ExternalInput").ap()
    logb = nc.dram_tensor("logb", [P, H], f32, kind="ExternalInput").ap()
    vecs = {}
    for name, used in (("bcb", has_bc), ("bgb", has_bg),
                       ("gammab", has_gamma), ("betab", has_beta)):
        if used:
            vecs[name] = nc.dram_tensor(name, [P, H], f32,
                                        kind="ExternalInput").ap()
    out = nc.dram_tensor("out", [BC, H], bft, kind="ExternalOutput").ap()

    w_specs = [("wcx8", wcx8, f8t), ("wgx8", wgx8, f8t),
               ("wgs8", wgs8, f8t), ("wcs", wcs, bft)]

    with tile.TileContext(nc) as tc, ExitStack() as ctx:
        singles = ctx.enter_context(tc.tile_pool(name="singles", bufs=1))
        actp = ctx.enter_context(tc.tile_pool(name="actp", bufs=1))
        wp = ctx.enter_context(tc.tile_pool(name="wp", bufs=2))
        psp = ctx.enter_context(tc.tile_pool(name="psp", bufs=3, space="PSUM"))
        epp = ctx.enter_context(tc.tile_pool(name="epp", bufs=2))
        stp = ctx.enter_context(tc.tile_pool(name="stp", bufs=2))
        hp = ctx.enter_context(tc.tile_pool(name="hp", bufs=1))
        statp = ctx.enter_context(tc.tile_pool(name="statp", bufs=1))
        normp = ctx.enter_context(tc.tile_pool(name="normp", bufs=4))
        outp = ctx.enter_context(tc.tile_pool(name="outp", bufs=2))

        # ---- startup-critical DMAs first: the g=0 matmuls need x8[0],
        # then the j=0 weight stream.  j=0 weights live in quarter-k
        # tiles (separate tiles, not just chunked DMAs) so the first
        # matmuls depend on 0.5 MB, not the full 4 MB; the x-path
        # weights stream before the s-path ones to match consumption
        # order.
        NCH = 4
        KCH = KT // NCH
        wt0q = {name: [wp.tile([P, KCH, NSL], dt, name=f"{name}_j0q{q}",
                               tag=f"{name}q{q}") for q in range(NCH)]
                for name, _, dt in w_specs}
        xs_t = [actp.tile([P, KT, P], f8t, name=f"x_g{g}", tag=f"x{g}")
                for g in range(G)]
        s8_t = [actp.tile([P, KT, P], f8t, name=f"s8_g{g}", tag=f"s8{g}")
                for g in range(G)]
        sb_t = [actp.tile([P, KT, P], bft, name=f"sb_g{g}", tag=f"sb{g}")
                for g in range(G)]
        for c in range(NCH):
            ksl = slice(c * KCH, (c + 1) * KCH)
            nc.sync.dma_start(out=xs_t[0][:, ksl], in_=x8[0][:, ksl])
        for c in range(NCH):
            ksl = slice(c * KCH, (c + 1) * KCH)
            for name in ("wcx8", "wgx8"):
                dram = dict((n, d) for n, d, _ in w_specs)[name]
                nc.sync.dma_start(out=wt0q[name][c][:], in_=dram[0][:, ksl])
            if c == 0:
                # g=0 s-path inputs, needed right after the x-path
                nc.sync.dma_start(out=sb_t[0][:], in_=sb[0])
                nc.sync.dma_start(out=s8_t[0][:], in_=s8[0])
        for c in range(NCH):
            ksl = slice(c * KCH, (c + 1) * KCH)
            for name in ("wgs8", "wcs"):
                dram = dict((n, d) for n, d, _ in w_specs)[name]
                nc.sync.dma_start(out=wt0q[name][c][:], in_=dram[0][:, ksl])

        # ---- constants: alpha = exp(-exp(-log_step)), broadcast [P, H] ----
        alpha_t = singles.tile([P, H], f32, name="alpha_t")
        nc.sync.dma_start(out=alpha_t[:, :H // 2], in_=logb[:, :H // 2])
        nc.sync.dma_start(out=alpha_t[:, H // 2:], in_=logb[:, H // 2:])
        nc.scalar.activation(alpha_t[:], alpha_t[:], AF.Exp, scale=-1.0)
        nc.scalar.activation(alpha_t[:], alpha_t[:], AF.Exp, scale=-1.0)
        c_one = singles.tile([P, 1], u32, name="c_one")
        nc.vector.memset(c_one[:], 1)
        vt = {}
        for name in vecs:
            vt[name] = singles.tile([P, H], f32, name=name + "_t")
            nc.sync.dma_start(out=vt[name][:], in_=vecs[name][:])

        # ---- per-group h accumulator (bf16) and layernorm stats ----
        h_t = [hp.tile([P, H], bft, name=f"h_g{g}", tag=f"h{g}")
               for g in range(G)]
        stats_t = [statp.tile([P, NJ, 6], f32, name=f"stats_g{g}", tag=f"st{g}")
                   for g in range(G)]

        # st slices are prefetched one (j, g) step ahead
        def st_fetch(j, g):
            t = stp.tile([P, NSL], bft, name=f"stsl_{j}_{g}", tag="stsl")
            nc.sync.dma_start(
                out=t[:], in_=st[g * P:(g + 1) * P, j * NSL:(j + 1) * NSL])
            return t

        st_next = st_fetch(0, 0)

        # ---- main loops: j = H slice, g = batch group ----
        for j in range(NJ):
            if j == 0:
                wtq = wt0q
            else:
                wtq = {}
                for name, dram, dt in w_specs:
                    tiles = [wp.tile([P, KCH, NSL], dt,
                                     name=f"{name}_j{j}q{q}",
                                     tag=f"{name}q{q}") for q in range(NCH)]
                    for q in range(NCH):
                        nc.sync.dma_start(
                            out=tiles[q][:],
                            in_=dram[j][:, q * KCH:(q + 1) * KCH])
                    wtq[name] = tiles

            def wk(name, k, n, wtq=wtq):
                q, lk = divmod(k, KCH)
                t = wtq[name][q]
                return t[:, lk, :] if n == 1 else t[:, lk:lk + n, :]
            jsl = slice(j * NSL, (j + 1) * NSL)

            def x_block(g, wk=wk):
                # x-path: fp8 DoubleRow, both branches share the
                # stationary x k-pair.  pcg holds cand preact in
                # [:, :NSL] and gate preact in [:, NSL:]
                pcg = psp.tile([P, 2 * NSL], f32, name=f"pcg_{j}_{g}",
                               tag="pcg")
                pc = pcg[:, :NSL]
                pg = pcg[:, NSL:]
                for kp in range(KP):
                    sl = slice(2 * kp, 2 * kp + 2)
                    nc.tensor.matmul(pc, xs_t[g][:, sl, :],
                                     wk("wcx8", 2 * kp, 2),
                                     start=(kp == 0), stop=False,
                                     perf_mode=DR)
                    nc.tensor.matmul(pg, xs_t[g][:, sl, :],
                                     wk("wgx8", 2 * kp, 2),
                                     start=(kp == 0), stop=False,
                                     perf_mode=DR)
                return pcg

            def s_block(g, pcg, wk=wk):
                # s-path: bf16 cand matmuls interleaved with fp8 DR gate
                # matmuls so the long DR weight loads hide under them
                pc = pcg[:, :NSL]
                pg = pcg[:, NSL:]
                for kp in range(KP):
                    k0, k1 = 2 * kp, 2 * kp + 1
                    last = kp == KP - 1
                    nc.tensor.matmul(pc, sb_t[g][:, k0, :],
                                     wk("wcs", k0, 1),
                                     start=False, stop=False)
                    nc.tensor.matmul(pc, sb_t[g][:, k1, :],
                                     wk("wcs", k1, 1),
                                     start=False, stop=last)
                    nc.tensor.matmul(pg, s8_t[g][:, slice(k0, k0 + 2), :],
                                     wk("wgs8", k0, 2),
                                     start=False, stop=last,
                                     perf_mode=DR)

            def epilogue(g, pcg, st_sl):
                # epilogue for this (g, j) slice; INV folds away the
                # fp8 quantization scales
                pc = pcg[:, :NSL]
                pg = pcg[:, NSL:]
                sc = epp.tile([P, NSL], f32, name=f"sc_{j}_{g}", tag="sc")
                sg = epp.tile([P, NSL], f32, name=f"sg_{j}_{g}", tag="sg")
                if has_bc:
                    nc.vector.scalar_tensor_tensor(
                        sc[:], pc, INV, vt["bcb"][:, jsl],
                        op0=OP.mult, op1=OP.add)
                    nc.scalar.activation(sc[:], sc[:], AF.Tanh)
                else:
                    nc.scalar.activation(sc[:], pc, AF.Tanh, scale=INV)
                if has_bg:
                    nc.vector.scalar_tensor_tensor(
                        sg[:], pg, INV, vt["bgb"][:, jsl],
                        op0=OP.mult, op1=OP.add)
                    nc.scalar.activation(sg[:], sg[:], AF.Sigmoid)
                else:
                    nc.scalar.activation(sg[:], pg, AF.Sigmoid, scale=INV)

                # h = gc + alpha*(state - gc), with gc = gate*cand;
                # reuse sg as gc and sc as the alpha term to save SBUF
                nc.vector.tensor_mul(sg[:], sc[:], sg[:])       # gc
                nc.vector.tensor_sub(sc[:], st_sl[:], sg[:])    # state-gc
                nc.vector.tensor_mul(sc[:], sc[:], alpha_t[:, jsl])
                # h slice lands directly in the bf16 accumulator
                nc.vector.tensor_add(h_t[g][:, jsl], sg[:], sc[:])

                nc.vector.bn_stats(out=stats_t[g][:, j, :],
                                   in_=h_t[g][:, jsl])

                if j == NJ - 1:
                    # layernorm + output for this group, overlapping the
                    # remaining groups' matmuls
                    mv = normp.tile([P, 2], f32, name=f"mv_{g}", tag="mv")
                    nc.vector.bn_aggr(out=mv[:], in_=stats_t[g][:])
                    # rstd = 1/sqrt(var+eps) entirely on the DVE (quake
                    # seed + 3 Newton steps) so ScalarE never swaps off
                    # the tanh/sigmoid activation table
                    vv = normp.tile([P, 1], f32, name=f"vv_{g}", tag="vv")
                    nc.vector.tensor_scalar_add(vv[:], mv[:, 1:2], EPS)
                    rstd = normp.tile([P, 1], f32, name=f"rstd_{g}",
                                      tag="rstd")
                    if DVE_RSQRT:
                        tf = normp.tile([P, 1], f32, name=f"tf_{g}",
                                        tag="tf")
                        tu = normp.tile([P, 1], u32, name=f"tu_{g}",
                                        tag="tu")
                        nc.vector.tensor_scalar(tu[:], vv[:].bitcast(u32),
                                                c_one[:], None,
                                                op0=OP.logical_shift_right)
                        nc.vector.tensor_copy(out=tf[:], in_=tu[:])
                        nc.vector.tensor_scalar(tf[:], tf[:], -1.0,
                                                float(0x5f3759df),
                                                op0=OP.mult, op1=OP.add)
                        nc.vector.tensor_copy(out=tu[:], in_=tf[:])
                        nc.vector.tensor_copy(out=rstd[:].bitcast(u32),
                                              in_=tu[:])
                        for _ in range(3):
                            nc.vector.tensor_mul(tf[:], rstd[:], rstd[:])
                            nc.vector.tensor_mul(tf[:], tf[:], vv[:])
                            nc.vector.tensor_scalar(tf[:], tf[:], -0.5, 1.5,
                                                    op0=OP.mult, op1=OP.add)
                            nc.vector.tensor_mul(rstd[:], rstd[:], tf[:])
                    else:
                        nc.scalar.activation(rstd[:], vv[:], AF.Sqrt)
                        nc.vector.reciprocal(rstd[:], rstd[:])
                    HH = H // 2
                    for half in range(2):
                        hs = slice(half * HH, (half + 1) * HH)
                        ot = outp.tile([P, HH], bft, name=f"ot_{g}_{half}",
                                       tag="ot")
                        nc.vector.tensor_scalar(ot[:], h_t[g][:, hs],
                                                mv[:, 0:1], rstd[:],
                                                op0=OP.subtract, op1=OP.mult)
                        if has_gamma:
                            nc.vector.tensor_mul(ot[:], ot[:],
                                                 vt["gammab"][:, hs])
                        if has_beta:
                            nc.vector.tensor_add(ot[:], ot[:],
                                                 vt["betab"][:, hs])
                        # split the store so the tail DMA pipelines
                        QH = HH // 2
                        for q in range(2):
                            cs = slice(half * HH + q * QH,
                                       half * HH + (q + 1) * QH)
                            nc.sync.dma_start(
                                out=out[g * P:(g + 1) * P, cs],
                                in_=ot[:, q * QH:(q + 1) * QH])

            def load_acts(g):
                # g>=1 activation loads, issued just before the x_block
                # that consumes them (j=0 only; tiles persist across j)
                nc.sync.dma_start(out=xs_t[g][:], in_=x8[g])
                nc.sync.dma_start(out=sb_t[g][:], in_=sb[g])
                nc.sync.dma_start(out=s8_t[g][:], in_=s8[g])

            # j=0: prefill 3 groups' x-paths (they only need the 2 MB of
            # fp8 x-weights) while the 3 MB of s-path weights stream in
            PREF = 2 if j == 0 else 0
            pcg_live = {}
            for g in range(PREF):
                if g >= 1:
                    load_acts(g)
                pcg_live[g] = x_block(g)
            for g in range(G):
                if g + PREF < G:
                    if j == 0 and g + PREF >= 1:
                        load_acts(g + PREF)
                    pcg_live[g + PREF] = x_block(g + PREF)
                pcg = pcg_live.pop(g)
                s_block(g, pcg)
                st_sl = st_next
                if not (j == NJ - 1 and g == G - 1):
                    nj, ng = (j, g + 1) if g + 1 < G else (j + 1, 0)
                    st_next = st_fetch(nj, ng)
                epilogue(g, pcg, st_sl)

    nc.compile()
    return nc


def _get_compiled(flags):
    if flags not in _compiled:
        _compiled[flags] = _build(flags)
    return _compiled[flags]


def kernel(x_t, state, Wc, Uc, bc, Wg, Ug, bg, log_step, gamma, beta):
    global LAST_RESULTS
    from concourse import bass_utils

    x_t = np.asarray(x_t, np.float32)
    state = np.asarray(state, np.float32)
    Wc = np.asarray(Wc, np.float32)
    Uc = np.asarray(Uc, np.float32)
    Wg = np.asarray(Wg, np.float32)
    Ug = np.asarray(Ug, np.float32)
    bc = np.asarray(bc, np.float32)
    bg = np.asarray(bg, np.float32)
    log_step = np.asarray(log_step, np.float32)
    gamma = np.asarray(gamma, np.float32)
    beta = np.asarray(beta, np.float32)

    def q8(a):
        return np.clip(a, -240.0, 240.0).astype(f8e4)

    # fold the recurrent weights, scale + cast, pre-tile for the device:
    # [j, p, k, n] = W[k*128+p, j*NSL+n]
    def wtile(w):
        return np.ascontiguousarray(
            w.reshape(KT, P, NJ, NSL).transpose(2, 1, 0, 3))

    w_maps = {
        "wcx8": wtile(q8(Wc[:IN] * SW)),
        "wgx8": wtile(q8(Wg[:IN] * SW)),
        "wgs8": wtile(q8((Wg[IN:] + Ug) * SW)),
        "wcs": wtile(((Wc[IN:] + Uc) * SW).astype(bf16)),
    }
    logb = np.ascontiguousarray(
        np.broadcast_to(log_step.reshape(1, H), (P, H)))

    flags = (bool(bc.any()), bool(bg.any()),
             bool((gamma != 1.0).any()), bool(beta.any()))
    vec_maps = {}
    if flags[0]:
        vec_maps["bcb"] = np.ascontiguousarray(
            np.broadcast_to(bc.reshape(1, H), (P, H)))
    if flags[1]:
        vec_maps["bgb"] = np.ascontiguousarray(
            np.broadcast_to(bg.reshape(1, H), (P, H)))
    if flags[2]:
        vec_maps["gammab"] = np.ascontiguousarray(
            np.broadcast_to(gamma.reshape(1, H), (P, H)))
    if flags[3]:
        vec_maps["betab"] = np.ascontiguousarray(
            np.broadcast_to(beta.reshape(1, H), (P, H)))

    nc = _get_compiled(flags)

    # per-core activation shards, pre-tiled: [g, p, k, m] = a[g*128+m, k*128+p]
    def atile(a):
        return np.ascontiguousarray(a.reshape(G, P, KT, P).transpose(0, 3, 2, 1))

    in_maps = []
    for c in range(NCORES):
        rows = slice(c * BC, (c + 1) * BC)
        xs = x_t[rows] * SX
        ss = state[rows] * SX
        m = {
            "x8": atile(q8(xs)),
            "s8": atile(q8(ss)),
            "sb": atile(ss.astype(bf16)),
            "st": np.ascontiguousarray(state[rows].astype(bf16)),
            "logb": logb,
        }
        m.update(w_maps)
        m.update(vec_maps)
        in_maps.append(m)

    trace_kwargs = {}
    if TRACE:
        trace_kwargs["trace_cores"] = list(range(NCORES))
    res = bass_utils.run_bass_kernel_spmd(
        nc, in_maps, core_ids=list(range(NCORES)), trace=TRACE,
        **trace_kwargs)
    LAST_RESULTS = res
    return np.concatenate(
        [res.results[c]["out"].astype(np.float32) for c in range(NCORES)],
        axis=0)
